# revision 1
# baseline (speedup 1.0000x reference)
"""GAT (3-layer, PyG GATConv-style, single head) on 8 Trainium2 NeuronCores.

Strategy (edge/graph parallel, dst-sharded):
  - Nodes are dealt to the 8 cores degree-serpentine (edge-balanced), then
    sorted within each core by (#A-side in-edges, total degree) into
    128-node blocks so per-block chunk schedules are tight.
  - Within a block, edge slot (p, g) holds an in-edge of node p.  With this
    layout the segment softmax is a plain free-dim reduction and the
    weighted scatter-aggregate is a PSUM accumulation of identity-weight
    matmuls.
  - Per layer each core computes the augmented node-table rows
    [h (64) | es = h@a_src | ed = h@a_dst | pad] (512B rows) for its own
    nodes; the full table is AllGather'd; per-edge rows are fetched with
    dma_gather (int16 indices -> two overlapping 32K-row windows: A = cores
    0-4, B = cores 5-7).
  - ed per destination node comes from a resident SBUF tile written by the
    previous layer's epilogue (never gathered).
"""

import os
import numpy as np

P = 128
NCORES = 8
DIN, HID, DOUT = 128, 64, 64
NEG_SLOPE = 0.2
ROW = 128          # table row elements (512B rows): h(64) | es | ed | pad
WIN = 1 << 15      # dma_gather int16 index window (rows)

_CACHE = {}
LAST_EXEC_NS = None
LAST_RESULT = None


# ----------------------------------------------------------------------------
# Host-side preprocessing (integer / layout work only)
# ----------------------------------------------------------------------------
def _preprocess(x, edge_index):
    N = x.shape[0]
    E = edge_index.shape[1]
    src = np.concatenate([np.asarray(edge_index[0]), np.arange(N)]).astype(np.int64)
    dst = np.concatenate([np.asarray(edge_index[1]), np.arange(N)]).astype(np.int64)
    n_edges = E + N

    deg = np.bincount(dst, minlength=N)  # >= 1 everywhere (self loops)

    # ---- step 1: deal nodes to cores, degree-serpentine (edge balance) ----
    by_deg = np.argsort(-deg, kind="stable")
    dev_of_node = np.empty(N, dtype=np.int64)
    for d in range(NCORES):
        dev_of_node[by_deg[d::NCORES]] = d
    n_per_dev = np.bincount(dev_of_node, minlength=NCORES)
    BPD = int(-(-n_per_dev.max() // P))
    SLICE = BPD * P
    NPAD = NCORES * SLICE

    # Source classes by table row: A window = [0, WIN), B = [NPAD-WIN, NPAD).
    # Rows in the overlap may be fetched by either gather.
    b_base = max(NPAD - WIN, 0)
    hi_base = min(WIN, NPAD)
    # node class depends only on which core it lives on and its within-core
    # rank, i.e. on the slot — but slots within a core are a permutation of
    # that core's nodes, so class-by-core-row-range is slot-independent only
    # at core granularity.  Classify conservatively by core:
    a_capable = (np.arange(NCORES) + 1) * SLICE <= hi_base
    b_capable = np.arange(NCORES) * SLICE >= b_base
    assert (a_capable | b_capable).all()
    cls_of_dev = np.where(a_capable & b_capable, 1, np.where(a_capable, 0, 2))
    src_cls = cls_of_dev[dev_of_node[src]]  # 0=A-only, 1=free, 2=B-only

    fAo = np.bincount(dst[src_cls == 0], minlength=N)
    fFr = np.bincount(dst[src_cls == 1], minlength=N)
    fBo = np.bincount(dst[src_cls == 2], minlength=N)

    # ---- step 2: within each core sort nodes by (deg, fAonly) ----
    node_of_slot = np.full(NPAD, -1, dtype=np.int64)
    slot_of_node = np.full(N, -1, dtype=np.int64)
    for d in range(NCORES):
        mine = np.where(dev_of_node == d)[0]
        order = mine[np.lexsort((-fAo[mine], -deg[mine]))]
        slots = d * SLICE + np.arange(len(order))
        node_of_slot[slots] = order
        slot_of_node[order] = slots

    li_of_slot = (np.arange(NPAD) % SLICE) // P

    # ---- step 3: per-level optimal shared (DA, DB) schedule ----
    fAo_s = np.zeros(NPAD, dtype=np.int64)
    fFr_s = np.zeros(NPAD, dtype=np.int64)
    fBo_s = np.zeros(NPAD, dtype=np.int64)
    real = node_of_slot >= 0
    fAo_s[real] = fAo[node_of_slot[real]]
    fFr_s[real] = fFr[node_of_slot[real]]
    fBo_s[real] = fBo[node_of_slot[real]]
    deg_s = fAo_s + fFr_s + fBo_s

    DA_sched = np.zeros(BPD, dtype=np.int64)
    DB_sched = np.zeros(BPD, dtype=np.int64)
    for li in range(BPD):
        m = li_of_slot == li
        FA, FR, FB, DG = fAo_s[m], fFr_s[m], fBo_s[m], deg_s[m]
        best = None
        for DAc in range(max(int(FA.max()), 1), int((FA + FR).max()) + 2):
            DBc = int(np.maximum(FB, DG - DAc).max())
            if best is None or DAc + DBc < best[0] + best[1]:
                best = (DAc, DBc)
        DA_sched[li], DB_sched[li] = best
    nA = np.minimum(fAo_s + fFr_s, DA_sched[li_of_slot])
    nB = deg_s - nA
    assert (nA <= DA_sched[li_of_slot]).all()
    assert (nB <= DB_sched[li_of_slot]).all()
    DT_sched = DA_sched + DB_sched
    OFF = np.concatenate([[0], np.cumsum(DT_sched)]).astype(np.int64)
    TOTD = int(OFF[-1])

    # ---- step 4: per-edge placement into (core, block, partition, slot) ----
    dslot = slot_of_node[dst]
    # order each node's edges [A-only, free, B-only]; first nA edges -> A
    grp_key = dslot * 4 + src_cls
    ord_e = np.argsort(grp_key, kind="stable")
    sd = dslot[ord_e]
    first = np.searchsorted(sd, sd, side="left")
    k_within = np.arange(n_edges) - first

    e_dev = sd // SLICE
    e_li = (sd % SLICE) // P
    e_p = sd % P
    e_isa = k_within < nA[sd]
    e_src_slot = slot_of_node[src[ord_e]]
    kA = k_within
    kB = k_within - nA[sd]
    assert (kA[e_isa] < DA_sched[e_li[e_isa]]).all()
    assert (kB[~e_isa] < DB_sched[e_li[~e_isa]]).all()

    # gather index value (window-relative)
    idx_val = np.where(e_isa, e_src_slot, e_src_slot - b_base)
    assert (idx_val >= 0).all() and (idx_val < WIN).all(), \
        (idx_val.min(), idx_val.max())

    # column within the block's [A chunks | B chunks] strip
    col = np.where(e_isa, kA, DA_sched[e_li] + kB)

    # idx arrays [core][128, TOTD] int16, default 0 (gathers a valid row,
    # contribution masked out).  Never negative: the Q7 trims trailing
    # negatives and mid-stream negatives generate wild DMA addresses.
    idx_all = np.zeros((NCORES, P, TOTD), dtype=np.int16)
    idx_all[e_dev, e_p, OFF[e_li] + col] = idx_val.astype(np.int16)
    assert (idx_all >= 0).all()

    # degree columns for masks [core][128, BPD]
    degA_c = np.zeros((NCORES, P, BPD), dtype=np.float32)
    degB_c = np.zeros((NCORES, P, BPD), dtype=np.float32)
    sl = np.arange(NPAD)
    degA_c[sl // SLICE, sl % P, li_of_slot] = nA
    degB_c[sl // SLICE, sl % P, li_of_slot] = nB

    # int16 gather-index stream in dma_gather layout:
    # per (block, group) strip, flat position i = g*128 + p ->
    # idx16[i % 16, i // 16], replicated down all 128 partitions.
    n16 = (TOTD * P) // 16
    idx16 = np.zeros((NCORES, P, n16), dtype=np.int16)
    for d in range(NCORES):
        cursor = 0
        for li in range(BPD):
            for (g0, gn) in ((0, int(DA_sched[li])),
                             (int(DA_sched[li]), int(DB_sched[li]))):
                if gn == 0:
                    continue
                blk = idx_all[d][:, OFF[li] + g0:OFF[li] + g0 + gn]  # [128,gn]
                flat = blk.T.reshape(-1)                  # i = g*128 + p
                wrapped = flat.reshape(-1, 16).T          # [16, gn*8]
                idx16[d][:, cursor:cursor + gn * 8] = np.tile(wrapped, (8, 1))
                cursor += gn * 8
        assert cursor == n16

    # full transposed features in slot order + per-core own slice
    xT = np.zeros((x.shape[1], NPAD), dtype=np.float32)
    real = node_of_slot >= 0
    xT[:, real] = np.asarray(x, dtype=np.float32)[node_of_slot[real]].T

    return dict(N=N, NPAD=NPAD, BPD=BPD, SLICE=SLICE,
                DA=[int(v) for v in DA_sched], DB=[int(v) for v in DB_sched],
                TOTD=TOTD, idx16=idx16, degA=degA_c, degB=degB_c, xT=xT,
                node_of_slot=node_of_slot,
                pad_ratio=float(TOTD * P * NCORES) / n_edges)


# ----------------------------------------------------------------------------
# Device program
# ----------------------------------------------------------------------------
def _build_program(NPAD, BPD, DA, DB, TOTD, n_layers=3, local_tables=False,
                   debug_stage=0, repeat=1):
    import concourse.bacc as bacc
    import concourse.tile as tile
    from concourse import mybir

    f32 = mybir.dt.float32
    i16 = mybir.dt.int16
    Alu = mybir.AluOpType
    Act = mybir.ActivationFunctionType
    X = mybir.AxisListType.X
    SLICE = BPD * P
    NSTRAT = NPAD // P
    OFF = np.concatenate([[0], np.cumsum(np.asarray(DA) + np.asarray(DB))])
    N16 = (TOTD * P) // 16
    b_base = max(NPAD - WIN, 0)

    nc = bacc.Bacc("TRN2", target_bir_lowering=False, debug=False,
                   num_devices=NCORES, num_swdge_queues=4)

    xT_d = nc.dram_tensor("xT", [DIN, NPAD], f32, kind="ExternalInput").ap()
    xTo_d = nc.dram_tensor("xTo", [DIN, SLICE], f32, kind="ExternalInput").ap()
    idx_d = nc.dram_tensor("idx16", [P, N16], i16, kind="ExternalInput").ap()
    degA_d = nc.dram_tensor("degA", [P, BPD], f32, kind="ExternalInput").ap()
    degB_d = nc.dram_tensor("degB", [P, BPD], f32, kind="ExternalInput").ap()
    ident_d = nc.dram_tensor("ident", [P, P], f32, kind="ExternalInput").ap()
    iota_d = nc.dram_tensor("iota", [P, P], f32, kind="ExternalInput").ap()
    w_d = [nc.dram_tensor(f"w{k}", [DIN if k == 0 else HID, ROW], f32,
                          kind="ExternalInput").ap() for k in range(3)]
    out_d = nc.dram_tensor("out", [SLICE, DOUT], f32, kind="ExternalOutput").ap()

    with tile.TileContext(nc) as tc:
        with tc.tile_pool(name="const", bufs=1) as cpool, \
             tc.tile_pool(name="dram", bufs=1, space="DRAM") as dpool, \
             tc.tile_pool(name="gin", bufs=4) as gin, \
             tc.tile_pool(name="raw", bufs=3) as rawp, \
             tc.tile_pool(name="smx", bufs=3) as smx, \
             tc.tile_pool(name="vv", bufs=6) as vv, \
             tc.tile_pool(name="ep", bufs=3) as ep, \
             tc.tile_pool(name="ps", bufs=2, space="PSUM") as ps:

            table0 = dpool.tile([NPAD, ROW], f32, name="table0")

            def alloc_tables(rep):
                table1s = dpool.tile([NPAD, ROW], f32, addr_space="Shared",
                                     name=f"table1s_{rep}")
                table2s = dpool.tile([NPAD, ROW], f32, addr_space="Shared",
                                     name=f"table2s_{rep}")
                if local_tables:
                    table1 = dpool.tile([NPAD, ROW], f32, name=f"table1_{rep}")
                    table2 = dpool.tile([NPAD, ROW], f32, name=f"table2_{rep}")
                else:
                    table1, table2 = table1s, table2s
                slice1 = dpool.tile([SLICE, ROW], f32, name=f"slice1_{rep}")
                slice2 = dpool.tile([SLICE, ROW], f32, name=f"slice2_{rep}")
                return ([table0, table1, table2], [None, table1s, table2s],
                        [None, slice1, slice2])

            ident_t = cpool.tile([P, P], f32, name="ident_t")
            nc.sync.dma_start(out=ident_t[:, :], in_=ident_d)
            iota_t = cpool.tile([P, P], f32, name="iota_t")
            nc.sync.dma_start(out=iota_t[:, :], in_=iota_d)
            w_t = []
            for k in range(3):
                wt = cpool.tile([DIN if k == 0 else HID, ROW], f32,
                                name=f"w_t{k}")
                nc.sync.dma_start(out=wt[:, :], in_=w_d[k])
                w_t.append(wt)
            idx_t = cpool.tile([P, N16], i16, name="idx_t")
            nc.sync.dma_start(out=idx_t[:, :], in_=idx_d)
            degA_t = cpool.tile([P, BPD], f32, name="degA_t")
            nc.sync.dma_start(out=degA_t[:, :], in_=degA_d)
            degB_t = cpool.tile([P, BPD], f32, name="degB_t")
            nc.sync.dma_start(out=degB_t[:, :], in_=degB_d)

            ed_res = cpool.tile([P, BPD], f32, name="ed_res")

            # 0/1 masks per edge slot, resident across layers
            mask_t = cpool.tile([P, TOTD], f32, name="mask_t")
            for li in range(BPD):
                o0 = int(OFF[li])
                if DA[li] > 0:
                    nc.vector.tensor_scalar(
                        out=mask_t[:, o0:o0 + DA[li]],
                        in0=iota_t[:, 0:DA[li]],
                        scalar1=degA_t[:, li:li + 1],
                        scalar2=None, op0=Alu.is_lt)
                if DB[li] > 0:
                    nc.vector.tensor_scalar(
                        out=mask_t[:, o0 + DA[li]:o0 + DA[li] + DB[li]],
                        in0=iota_t[:, 0:DB[li]],
                        scalar1=degB_t[:, li:li + 1],
                        scalar2=None, op0=Alu.is_lt)

            # ---- layer-1 table: replicated GEMM over all strata ----
            def one_pass(rep):
              tables, tables_sh, slices = alloc_tables(rep)
              for j in range(NSTRAT):
                xt = gin.tile([DIN, P], f32, tag="xt")
                nc.sync.dma_start(out=xt[:, :], in_=xT_d[:, j * P:(j + 1) * P])
                pst = ps.tile([P, ROW], f32, tag="ptab")
                nc.tensor.matmul(out=pst[:, :], lhsT=xt[:, :], rhs=w_t[0][:, :],
                                 start=True, stop=True)
                tt = gin.tile([P, ROW], f32, tag="tabt")
                nc.scalar.activation(out=tt[:, :], in_=pst[:, :], func=Act.Copy)
                nc.sync.dma_start(out=table0[j * P:(j + 1) * P, :],
                                  in_=tt[:, :])

              # layer-1 ed for own nodes
              for li in range(BPD):
                xt = gin.tile([DIN, P], f32, tag="xt")
                nc.sync.dma_start(out=xt[:, :], in_=xTo_d[:, li * P:(li + 1) * P])
                pse = ps.tile([P, 2], f32, tag="ped")
                nc.tensor.matmul(out=pse[:, :], lhsT=xt[:, :],
                                 rhs=w_t[0][:, 64:66], start=True, stop=True)
                nc.vector.tensor_copy(out=ed_res[:, li:li + 1],
                                      in_=pse[:, 1:2])

              # ---- layers ----
              for k in range(n_layers):
                table_full = tables[k]
                if k > 0:
                    nc.gpsimd.collective_compute(
                        "AllGather", Alu.bypass,
                        replica_groups=[list(range(NCORES))],
                        ins=[slices[k][:, :]], outs=[tables_sh[k][:, :]])
                    if local_tables:
                        nc.sync.dma_start(out=table_full[:, :],
                                          in_=tables_sh[k][:, :])
                i16cur = 0
                qrr = 0
                for li in range(BPD):
                    DAl, DBl = DA[li], DB[li]
                    DT = DAl + DBl
                    o0 = int(OFF[li])
                    raw = rawp.tile([P, DT, ROW], f32, tag="raw")
                    if debug_stage == 5:
                        nc.vector.tensor_scalar(
                            out=raw[:, :, :], in0=raw[:, :, :], scalar1=0.0,
                            scalar2=None, op0=Alu.mult)
                    for (gbase, gn, wbase) in ((0, DAl, 0), (DAl, DBl, b_base)):
                        if gn == 0 or debug_stage == 5:
                            continue
                        nidx = gn * P
                        nc.gpsimd.dma_gather(
                            out_ap=raw[:, gbase:gbase + gn, :],
                            in_ap=table_full[wbase:min(wbase + WIN, NPAD), :],
                            idxs_ap=idx_t[:, i16cur:i16cur + nidx // 16],
                            num_idxs=nidx, num_idxs_reg=nidx,
                            elem_size=ROW, single_packet=(nidx <= 1024),
                            queue_num=qrr % 4)
                        qrr += 1
                        i16cur += nidx // 16
                    if debug_stage == 1:
                        dbg = ep.tile([P, DOUT], f32, tag="outf")
                        nc.vector.tensor_copy(out=dbg[:, :],
                                              in_=raw[:, 0, 0:DOUT])
                        nc.sync.dma_start(out=out_d[li * P:(li + 1) * P, :],
                                          in_=dbg[:, :])
                        continue
                    es = raw[:, :, 64]
                    ed = ed_res[:, li:li + 1]
                    t_t = smx.tile([P, DT], f32, tag="t")
                    nc.vector.tensor_scalar(out=t_t[:, :], in0=es,
                                            scalar1=ed, scalar2=None,
                                            op0=Alu.add)
                    q_t = smx.tile([P, DT], f32, tag="q")
                    nc.vector.tensor_scalar(out=q_t[:, :], in0=t_t[:, :],
                                            scalar1=NEG_SLOPE, scalar2=None,
                                            op0=Alu.mult)
                    lk = smx.tile([P, DT], f32, tag="lk")
                    nc.vector.tensor_tensor(out=lk[:, :], in0=t_t[:, :],
                                            in1=q_t[:, :], op=Alu.max)
                    pr = smx.tile([P, DT], f32, tag="pr")
                    nc.scalar.activation(out=pr[:, :], in_=lk[:, :],
                                         func=Act.Exp)
                    pm = smx.tile([P, DT], f32, tag="pm")
                    nc.vector.tensor_tensor(out=pm[:, :], in0=pr[:, :],
                                            in1=mask_t[:, o0:o0 + DT],
                                            op=Alu.mult)
                    s_t = smx.tile([P, 1], f32, tag="s")
                    nc.vector.tensor_reduce(out=s_t[:, :], in_=pm[:, :],
                                            axis=X, op=Alu.add)
                    se = smx.tile([P, 1], f32, tag="se")
                    nc.vector.tensor_scalar(out=se[:, :], in0=s_t[:, :],
                                            scalar1=1e-16, scalar2=None,
                                            op0=Alu.add)
                    r_t = smx.tile([P, 1], f32, tag="r")
                    nc.vector.reciprocal(out=r_t[:, :], in_=se[:, :])

                    if debug_stage == 2:
                        dbg = ep.tile([P, DOUT], f32, tag="outf")
                        nc.vector.tensor_scalar(out=dbg[:, :],
                                                in0=iota_t[:, 0:DOUT],
                                                scalar1=r_t[:, :],
                                                scalar2=None, op0=Alu.mult)
                        nc.sync.dma_start(out=out_d[li * P:(li + 1) * P, :],
                                          in_=dbg[:, :])
                        continue

                    agg = ps.tile([P, DOUT], f32, tag="agg")
                    for g in range(DT):
                        v_t = vv.tile([P, DOUT], f32, tag="v")
                        nc.vector.tensor_scalar(out=v_t[:, :],
                                                in0=raw[:, g, 0:DOUT],
                                                scalar1=pm[:, g:g + 1],
                                                scalar2=None, op0=Alu.mult)
                        nc.tensor.matmul(out=agg[:, :], lhsT=ident_t[:, :],
                                         rhs=v_t[:, :], start=(g == 0),
                                         stop=(g == DT - 1))

                    if k < n_layers - 1:
                        outb = ep.tile([P, DOUT], f32, tag="outb")
                        nc.scalar.activation(out=outb[:, :], in_=agg[:, :],
                                             func=Act.Relu, scale=r_t[:, :])
                        ptr = ps.tile([HID, P], f32, tag="ptr")
                        nc.tensor.transpose(out=ptr[:, :], in_=outb[:, :],
                                            identity=ident_t[:, :])
                        xtb = ep.tile([HID, P], f32, tag="xtb")
                        nc.vector.tensor_copy(out=xtb[:, :], in_=ptr[:, :])
                        ptab = ps.tile([P, ROW], f32, tag="ptab")
                        nc.tensor.matmul(out=ptab[:, :], lhsT=xtb[:, :],
                                         rhs=w_t[k + 1][:, :], start=True,
                                         stop=True)
                        nc.vector.tensor_copy(out=ed_res[:, li:li + 1],
                                              in_=ptab[:, 65:66])
                        tabt = ep.tile([P, ROW], f32, tag="tabt2")
                        nc.scalar.activation(out=tabt[:, :], in_=ptab[:, :],
                                             func=Act.Copy)
                        nc.sync.dma_start(
                            out=slices[k + 1][li * P:(li + 1) * P, :],
                            in_=tabt[:, :])
                    else:
                        z_t = ep.tile([P, DOUT], f32, tag="z")
                        nc.scalar.activation(out=z_t[:, :], in_=agg[:, :],
                                             func=Act.Copy, scale=r_t[:, :])
                        if debug_stage == 7:
                            nc.sync.dma_start(
                                out=out_d[li * P:(li + 1) * P, :], in_=z_t[:, :])
                            continue
                        ez = ep.tile([P, DOUT], f32, tag="ez")
                        ssum = ep.tile([P, 1], f32, tag="ssum")
                        nc.scalar.activation(out=ez[:, :], in_=z_t[:, :],
                                             func=Act.Exp,
                                             accum_out=ssum[:, :])
                        ls = ep.tile([P, 1], f32, tag="ls")
                        nc.scalar.activation(out=ls[:, :], in_=ssum[:, :],
                                             func=Act.Ln)
                        outf = ep.tile([P, DOUT], f32, tag="outf")
                        nc.vector.tensor_scalar(out=outf[:, :], in0=z_t[:, :],
                                                scalar1=ls[:, :], scalar2=None,
                                                op0=Alu.subtract)
                        nc.sync.dma_start(out=out_d[li * P:(li + 1) * P, :],
                                          in_=outf[:, :])
                assert i16cur == N16 or debug_stage == 5

            for _rep in range(repeat):
                one_pass(_rep)

    nc.compile()
    return nc


# ----------------------------------------------------------------------------
# Entry point
# ----------------------------------------------------------------------------
def _make_inputs(pre, W_list):
    ws = []
    for (W, asr, ads) in W_list:
        W = np.asarray(W, dtype=np.float32)
        din = W.shape[0]
        waug = np.zeros((din, ROW), dtype=np.float32)
        waug[:, :64] = W
        waug[:, 64] = W @ np.asarray(asr, np.float32)
        waug[:, 65] = W @ np.asarray(ads, np.float32)
        ws.append(waug)
    ident = np.eye(P, dtype=np.float32)
    iota = np.broadcast_to(np.arange(P, dtype=np.float32), (P, P)).copy()
    SLICE = pre["SLICE"]
    in_maps = []
    for d in range(NCORES):
        in_maps.append({
            "xT": np.ascontiguousarray(pre["xT"]),
            "xTo": np.ascontiguousarray(pre["xT"][:, d * SLICE:(d + 1) * SLICE]),
            "idx16": np.ascontiguousarray(pre["idx16"][d]),
            "degA": np.ascontiguousarray(pre["degA"][d]),
            "degB": np.ascontiguousarray(pre["degB"][d]),
            "ident": ident, "iota": iota,
            "w0": ws[0], "w1": ws[1], "w2": ws[2],
        })
    return in_maps


def kernel(x, edge_index, W0, a_src0, a_dst0, W1, a_src1, a_dst1,
           W2, a_src2, a_dst2):
    global LAST_EXEC_NS, LAST_RESULT
    from concourse.bass_utils import run_bass_kernel_spmd

    x = np.asarray(x, dtype=np.float32)
    pre = _preprocess(x, np.asarray(edge_index))

    key = (pre["NPAD"], pre["BPD"], tuple(pre["DA"]), tuple(pre["DB"]))
    if key not in _CACHE:
        _CACHE[key] = _build_program(pre["NPAD"], pre["BPD"], pre["DA"],
                                     pre["DB"], pre["TOTD"])
    nc = _CACHE[key]

    in_maps = _make_inputs(pre, ((W0, a_src0, a_dst0), (W1, a_src1, a_dst1),
                                 (W2, a_src2, a_dst2)))
    trace = bool(int(os.environ.get("GAT_TRACE", "0")))
    res = run_bass_kernel_spmd(nc, in_maps, list(range(NCORES)), trace=trace)
    LAST_EXEC_NS = res.exec_time_ns
    LAST_RESULT = res

    out = np.zeros((pre["N"], DOUT), dtype=np.float32)
    SLICE = pre["SLICE"]
    for d in range(NCORES):
        od = res.results[d]["out"]
        nodes = pre["node_of_slot"][d * SLICE:(d + 1) * SLICE]
        ok = nodes >= 0
        out[nodes[ok]] = od[ok]
    return out



# revision 3
# speedup vs baseline: 98.8501x; 98.8501x over previous
"""GAT (3-layer, PyG GATConv-style, single head) on 8 Trainium2 NeuronCores.

v3: same edge/graph-parallel dst-sharded layout as the baseline, plus:
  - ALL per-core inputs packed into ONE [128, CBLOB] f32 tensor (per-call
    argument marshalling dominates the measured time; the baseline shipped
    ~32 MB across 11 args per core, v3 ships ~2 MB in one).
  - The per-layer node table is bf16 with 256B rows (h[64] | es_hi | es_lo |
    pad), halving gather bytes; es is carried as a bf16 hi/lo pair so the
    attention logits keep ~f32 accuracy.
  - Layer-0 no longer replicates the full-graph GEMM per core: each core
    computes its own slice's augmented rows and the table is AllGathered
    (3 small bf16 collectives per pass instead of 2 f32 ones + a 25.7 MB
    HBM stream).
  - int16 gather indices ship once as [16, N16] and are replicated to 128
    partitions on device.
  - GEMMs, aggregation matmuls and the output are bf16 (output upcast on
    host).
"""

import os
import numpy as np

P = 128
NCORES = 8
DIN, HID, DOUT = 128, 64, 64
NEG_SLOPE = 0.2
ROW = 128          # table row elements (bf16): h(64) | es_hi | es_lo | pad
WIN = 1 << 15      # dma_gather int16 index window (rows)

_CACHE = {}
LAST_EXEC_NS = None
LAST_RESULT = None


# ----------------------------------------------------------------------------
# Host-side preprocessing (integer / layout work only) — same as baseline
# ----------------------------------------------------------------------------
def _preprocess(x, edge_index):
    N = x.shape[0]
    E = edge_index.shape[1]
    src = np.concatenate([np.asarray(edge_index[0]), np.arange(N)]).astype(np.int64)
    dst = np.concatenate([np.asarray(edge_index[1]), np.arange(N)]).astype(np.int64)
    n_edges = E + N

    deg = np.bincount(dst, minlength=N)  # >= 1 everywhere (self loops)

    by_deg = np.argsort(-deg, kind="stable")
    dev_of_node = np.empty(N, dtype=np.int64)
    for d in range(NCORES):
        dev_of_node[by_deg[d::NCORES]] = d
    n_per_dev = np.bincount(dev_of_node, minlength=NCORES)
    BPD = int(-(-n_per_dev.max() // P))
    SLICE = BPD * P
    NPAD = NCORES * SLICE

    b_base = max(NPAD - WIN, 0)
    hi_base = min(WIN, NPAD)
    a_capable = (np.arange(NCORES) + 1) * SLICE <= hi_base
    b_capable = np.arange(NCORES) * SLICE >= b_base
    assert (a_capable | b_capable).all()
    cls_of_dev = np.where(a_capable & b_capable, 1, np.where(a_capable, 0, 2))
    src_cls = cls_of_dev[dev_of_node[src]]  # 0=A-only, 1=free, 2=B-only

    fAo = np.bincount(dst[src_cls == 0], minlength=N)
    fFr = np.bincount(dst[src_cls == 1], minlength=N)
    fBo = np.bincount(dst[src_cls == 2], minlength=N)

    node_of_slot = np.full(NPAD, -1, dtype=np.int64)
    slot_of_node = np.full(N, -1, dtype=np.int64)
    for d in range(NCORES):
        mine = np.where(dev_of_node == d)[0]
        order = mine[np.lexsort((-fAo[mine], -deg[mine]))]
        slots = d * SLICE + np.arange(len(order))
        node_of_slot[slots] = order
        slot_of_node[order] = slots

    li_of_slot = (np.arange(NPAD) % SLICE) // P

    fAo_s = np.zeros(NPAD, dtype=np.int64)
    fFr_s = np.zeros(NPAD, dtype=np.int64)
    fBo_s = np.zeros(NPAD, dtype=np.int64)
    real = node_of_slot >= 0
    fAo_s[real] = fAo[node_of_slot[real]]
    fFr_s[real] = fFr[node_of_slot[real]]
    fBo_s[real] = fBo[node_of_slot[real]]
    deg_s = fAo_s + fFr_s + fBo_s

    DA_sched = np.zeros(BPD, dtype=np.int64)
    DB_sched = np.zeros(BPD, dtype=np.int64)
    for li in range(BPD):
        m = li_of_slot == li
        FA, FR, FB, DG = fAo_s[m], fFr_s[m], fBo_s[m], deg_s[m]
        best = None
        for DAc in range(max(int(FA.max()), 1), int((FA + FR).max()) + 2):
            DBc = int(np.maximum(FB, DG - DAc).max())
            if best is None or DAc + DBc < best[0] + best[1]:
                best = (DAc, DBc)
        DA_sched[li], DB_sched[li] = best
    nA = np.minimum(fAo_s + fFr_s, DA_sched[li_of_slot])
    nB = deg_s - nA
    assert (nA <= DA_sched[li_of_slot]).all()
    assert (nB <= DB_sched[li_of_slot]).all()
    DT_sched = DA_sched + DB_sched
    OFF = np.concatenate([[0], np.cumsum(DT_sched)]).astype(np.int64)
    TOTD = int(OFF[-1])

    dslot = slot_of_node[dst]
    grp_key = dslot * 4 + src_cls
    ord_e = np.argsort(grp_key, kind="stable")
    sd = dslot[ord_e]
    first = np.searchsorted(sd, sd, side="left")
    k_within = np.arange(n_edges) - first

    e_dev = sd // SLICE
    e_li = (sd % SLICE) // P
    e_p = sd % P
    e_isa = k_within < nA[sd]
    e_src_slot = slot_of_node[src[ord_e]]
    kA = k_within
    kB = k_within - nA[sd]
    assert (kA[e_isa] < DA_sched[e_li[e_isa]]).all()
    assert (kB[~e_isa] < DB_sched[e_li[~e_isa]]).all()

    idx_val = np.where(e_isa, e_src_slot, e_src_slot - b_base)
    assert (idx_val >= 0).all() and (idx_val < WIN).all(), \
        (idx_val.min(), idx_val.max())

    col = np.where(e_isa, kA, DA_sched[e_li] + kB)

    idx_all = np.zeros((NCORES, P, TOTD), dtype=np.int16)
    idx_all[e_dev, e_p, OFF[e_li] + col] = idx_val.astype(np.int16)
    assert (idx_all >= 0).all()

    degA_c = np.zeros((NCORES, P, BPD), dtype=np.float32)
    degB_c = np.zeros((NCORES, P, BPD), dtype=np.float32)
    sl = np.arange(NPAD)
    degA_c[sl // SLICE, sl % P, li_of_slot] = nA
    degB_c[sl // SLICE, sl % P, li_of_slot] = nB

    # int16 gather-index stream, 16-partition wrapped (NOT replicated here;
    # the device replicates to 128 partitions)
    n16 = (TOTD * P) // 16
    idx16 = np.zeros((NCORES, 16, n16), dtype=np.int16)
    for d in range(NCORES):
        cursor = 0
        for li in range(BPD):
            for (g0, gn) in ((0, int(DA_sched[li])),
                             (int(DA_sched[li]), int(DB_sched[li]))):
                if gn == 0:
                    continue
                blk = idx_all[d][:, OFF[li] + g0:OFF[li] + g0 + gn]  # [128,gn]
                flat = blk.T.reshape(-1)                  # i = g*128 + p
                idx16[d][:, cursor:cursor + gn * 8] = flat.reshape(-1, 16).T
                cursor += gn * 8
        assert cursor == n16

    # transposed features in slot order
    xT = np.zeros((x.shape[1], NPAD), dtype=np.float32)
    real = node_of_slot >= 0
    xT[:, real] = np.asarray(x, dtype=np.float32)[node_of_slot[real]].T

    return dict(N=N, NPAD=NPAD, BPD=BPD, SLICE=SLICE,
                DA=[int(v) for v in DA_sched], DB=[int(v) for v in DB_sched],
                TOTD=TOTD, idx16=idx16, degA=degA_c, degB=degB_c, xT=xT,
                node_of_slot=node_of_slot,
                pad_ratio=float(TOTD * P * NCORES) / n_edges)


def _blob_layout(SLICE, BPD, TOTD):
    """Column offsets (f32 units) of each section in the packed input blob."""
    n16 = (TOTD * P) // 16
    NI16 = -(-n16 * 16 // 16)  # i16 per wrapped row = n16
    NI16 = n16
    # pad idx columns so NI16 % 16 == 0 (so NI16/8 i16 = NI16/16 f32 is whole)
    NI16p = ((NI16 + 15) // 16) * 16
    off = {}
    c = 0
    off["xTo"] = c; c += SLICE // 2          # bf16 [128, SLICE]
    off["idx"] = c; c += NI16p // 16 * 2     # i16 [128, NI16p/8]
    off["degA"] = c; c += BPD
    off["degB"] = c; c += BPD
    off["w0"] = c; c += ROW // 2             # bf16 [128, 128]
    off["w12"] = c; c += ROW                 # bf16 [64, 128]|[64, 128] in cols
    off["identb"] = c; c += P // 2           # bf16 [128, 128]
    off["iota"] = c; c += P                  # f32 [128, 128]
    return off, c, NI16p


# ----------------------------------------------------------------------------
# Device program
# ----------------------------------------------------------------------------
def _build_program(NPAD, BPD, DA, DB, TOTD, n_layers=3, repeat=1,
                   skip_coll=False, skip_gather=False):
    import concourse.bacc as bacc
    import concourse.tile as tile
    from concourse import mybir

    f32 = mybir.dt.float32
    bf16 = mybir.dt.bfloat16
    i16 = mybir.dt.int16
    Alu = mybir.AluOpType
    Act = mybir.ActivationFunctionType
    X = mybir.AxisListType.X
    SLICE = BPD * P
    OFF = np.concatenate([[0], np.cumsum(np.asarray(DA) + np.asarray(DB))])
    N16 = (TOTD * P) // 16
    b_base = max(NPAD - WIN, 0)
    offs, CBLOB, NI16p = _blob_layout(SLICE, BPD, TOTD)
    C2 = NI16p // 8   # i16 idx columns per partition in the blob

    nc = bacc.Bacc("TRN2", target_bir_lowering=False, debug=False,
                   num_devices=NCORES, num_swdge_queues=4)

    blob_d = nc.dram_tensor("blob", [P, CBLOB], f32, kind="ExternalInput").ap()
    out_d = nc.dram_tensor("out", [SLICE, DOUT], bf16, kind="ExternalOutput").ap()

    with tile.TileContext(nc) as tc:
        with tc.tile_pool(name="const", bufs=1) as cpool, \
             tc.tile_pool(name="dram", bufs=1, space="DRAM") as dpool, \
             tc.tile_pool(name="raw", bufs=4) as rawp, \
             tc.tile_pool(name="smx", bufs=3) as smx, \
             tc.tile_pool(name="vv", bufs=6) as vv, \
             tc.tile_pool(name="ep", bufs=3) as ep, \
             tc.tile_pool(name="pse", bufs=2, space="PSUM") as pse, \
             tc.tile_pool(name="psa", bufs=2, space="PSUM") as psa, \
             tc.tile_pool(name="pst", bufs=2, space="PSUM") as pst:

            bstage = cpool.tile([P, CBLOB], f32, name="bstage")
            nc.sync.dma_start(out=bstage[:, :], in_=blob_d)

            xTo_v = bstage[:, offs["xTo"]:offs["xTo"] + SLICE // 2].bitcast(bf16)
            degA_v = bstage[:, offs["degA"]:offs["degA"] + BPD]
            degB_v = bstage[:, offs["degB"]:offs["degB"] + BPD]
            w0_v = bstage[:, offs["w0"]:offs["w0"] + ROW // 2].bitcast(bf16)
            w1_v = bstage[0:HID,
                          offs["w12"]:offs["w12"] + ROW // 2].bitcast(bf16)
            w2_v = bstage[0:HID, offs["w12"] + ROW // 2:
                          offs["w12"] + ROW].bitcast(bf16)
            w_next = [None, w1_v, w2_v]
            identb_v = bstage[:, offs["identb"]:offs["identb"] + P // 2].bitcast(bf16)
            iota_v = bstage[:, offs["iota"]:offs["iota"] + P]

            # ---- idx16: assemble [16, NI16p] then replicate to 128 parts ----
            idx_t = cpool.tile([P, NI16p], i16, name="idx_t")
            for a in range(8):
                nc.sync.dma_start(
                    out=idx_t[0:16, a * C2:(a + 1) * C2],
                    in_=blob_d[16 * a:16 * (a + 1),
                               offs["idx"]:offs["idx"] + C2 // 2].bitcast(i16))
            for k in range(1, 8):
                nc.sync.dma_start(out=idx_t[16 * k:16 * (k + 1), :],
                                  in_=idx_t[0:16, :])

            ed_res = cpool.tile([P, BPD], f32, name="ed_res")

            # 0/1 masks per edge slot, resident across layers
            mask_t = cpool.tile([P, TOTD], f32, name="mask_t")
            for li in range(BPD):
                o0 = int(OFF[li])
                if DA[li] > 0:
                    nc.vector.tensor_scalar(
                        out=mask_t[:, o0:o0 + DA[li]],
                        in0=iota_v[:, 0:DA[li]],
                        scalar1=degA_v[:, li:li + 1],
                        scalar2=None, op0=Alu.is_lt)
                if DB[li] > 0:
                    nc.vector.tensor_scalar(
                        out=mask_t[:, o0 + DA[li]:o0 + DA[li] + DB[li]],
                        in0=iota_v[:, 0:DB[li]],
                        scalar1=degB_v[:, li:li + 1],
                        scalar2=None, op0=Alu.is_lt)

            def one_pass(rep):
                imgslice = [dpool.tile([SLICE, ROW], bf16, name=f"imgs{rep}_{k}")
                            for k in range(n_layers)]
                imgfull = [dpool.tile([NPAD, ROW], bf16, addr_space="Shared",
                                      name=f"imgf{rep}_{k}")
                           for k in range(n_layers)]

                def write_stripe(k, li, ps_tab):
                    """Augmented GEMM out (PSUM f32 [P, ROW]) -> packed bf16
                    image stripe + resident ed."""
                    img_t = ep.tile([P, ROW], bf16, tag="img")
                    nc.scalar.activation(out=img_t[:, 0:HID],
                                         in_=ps_tab[:, 0:HID], func=Act.Copy)
                    nc.vector.tensor_copy(out=img_t[:, 64:65],
                                          in_=ps_tab[:, 64:65])
                    hif = smx.tile([P, 1], f32, tag="hif")
                    nc.vector.tensor_copy(out=hif[:, :], in_=img_t[:, 64:65])
                    lof = smx.tile([P, 1], f32, tag="lof")
                    nc.vector.tensor_tensor(out=lof[:, :], in0=ps_tab[:, 64:65],
                                            in1=hif[:, :], op=Alu.subtract)
                    nc.vector.tensor_copy(out=img_t[:, 65:66], in_=lof[:, :])
                    nc.vector.tensor_copy(out=ed_res[:, li:li + 1],
                                          in_=ps_tab[:, 65:66])
                    nc.sync.dma_start(out=imgslice[k][li * P:(li + 1) * P, :],
                                      in_=img_t[:, :])

                def gather_table(k):
                    if skip_coll:
                        return
                    nc.gpsimd.collective_compute(
                        "AllGather", Alu.bypass,
                        replica_groups=[list(range(NCORES))],
                        ins=[imgslice[k][:, :]], outs=[imgfull[k][:, :]])

                # ---- layer-0 slice GEMM ----
                for li in range(BPD):
                    ps_tab = pse.tile([P, ROW], f32, tag="pse")
                    nc.tensor.matmul(out=ps_tab[:, :],
                                     lhsT=xTo_v[:, li * P:(li + 1) * P],
                                     rhs=w0_v[:, :], start=True, stop=True)
                    write_stripe(0, li, ps_tab)
                gather_table(0)

                # ---- layers ----
                for k in range(n_layers):
                    table = imgfull[k]
                    i16cur = 0
                    qrr = 0
                    for li in range(BPD):
                        DAl, DBl = DA[li], DB[li]
                        DT = DAl + DBl
                        o0 = int(OFF[li])
                        raw = rawp.tile([P, DT, ROW], bf16, tag="raw")
                        for (gbase, gn, wbase) in ((0, DAl, 0),
                                                   (DAl, DBl, b_base)):
                            if gn == 0:
                                continue
                            if skip_gather:
                                i16cur += gn * P // 16
                                continue
                            nidx = gn * P
                            nc.gpsimd.dma_gather(
                                out_ap=raw[:, gbase:gbase + gn, :],
                                in_ap=table[wbase:min(wbase + WIN, NPAD), :],
                                idxs_ap=idx_t[:, i16cur:i16cur + nidx // 16],
                                num_idxs=nidx, num_idxs_reg=nidx,
                                elem_size=ROW, single_packet=(nidx <= 1024),
                                queue_num=qrr % 4)
                            qrr += 1
                            i16cur += nidx // 16

                        # es = hi + lo; t = es + ed; leakyrelu; exp; mask
                        t_t = smx.tile([P, DT], f32, tag="t")
                        nc.vector.tensor_tensor(out=t_t[:, :],
                                                in0=raw[:, :, 64],
                                                in1=raw[:, :, 65], op=Alu.add)
                        t2 = smx.tile([P, DT], f32, tag="t2")
                        nc.vector.tensor_scalar(out=t2[:, :], in0=t_t[:, :],
                                                scalar1=ed_res[:, li:li + 1],
                                                scalar2=None, op0=Alu.add)
                        q_t = smx.tile([P, DT], f32, tag="q")
                        nc.vector.tensor_scalar(out=q_t[:, :], in0=t2[:, :],
                                                scalar1=NEG_SLOPE, scalar2=None,
                                                op0=Alu.mult)
                        lk = smx.tile([P, DT], f32, tag="lk")
                        nc.vector.tensor_tensor(out=lk[:, :], in0=t2[:, :],
                                                in1=q_t[:, :], op=Alu.max)
                        pr = smx.tile([P, DT], f32, tag="pr")
                        nc.scalar.activation(out=pr[:, :], in_=lk[:, :],
                                             func=Act.Exp)
                        pm = smx.tile([P, DT], f32, tag="pm")
                        nc.vector.tensor_tensor(out=pm[:, :], in0=pr[:, :],
                                                in1=mask_t[:, o0:o0 + DT],
                                                op=Alu.mult)
                        s_t = smx.tile([P, 1], f32, tag="s")
                        nc.vector.tensor_reduce(out=s_t[:, :], in_=pm[:, :],
                                                axis=X, op=Alu.add)
                        se = smx.tile([P, 1], f32, tag="se")
                        nc.vector.tensor_scalar(out=se[:, :], in0=s_t[:, :],
                                                scalar1=1e-16, scalar2=None,
                                                op0=Alu.add)
                        r_t = smx.tile([P, 1], f32, tag="r")
                        nc.vector.reciprocal(out=r_t[:, :], in_=se[:, :])

                        agg = psa.tile([P, DOUT], f32, tag="agg")
                        for g in range(DT):
                            v_t = vv.tile([P, DOUT], bf16, tag="v")
                            nc.vector.tensor_scalar(out=v_t[:, :],
                                                    in0=raw[:, g, 0:HID],
                                                    scalar1=pm[:, g:g + 1],
                                                    scalar2=None, op0=Alu.mult)
                            nc.tensor.matmul(out=agg[:, :],
                                             lhsT=identb_v[:, :],
                                             rhs=v_t[:, :], start=(g == 0),
                                             stop=(g == DT - 1))

                        if k < n_layers - 1:
                            outb = ep.tile([P, DOUT], bf16, tag="outb")
                            nc.scalar.activation(out=outb[:, :], in_=agg[:, :],
                                                 func=Act.Relu, scale=r_t[:, :])
                            ptr = pst.tile([HID, P], bf16, tag="ptr")
                            nc.tensor.transpose(out=ptr[:, :], in_=outb[:, :],
                                                identity=identb_v[:, :])
                            xtb = ep.tile([HID, P], bf16, tag="xtb")
                            nc.vector.tensor_copy(out=xtb[:, :], in_=ptr[:, :])
                            ptab = pse.tile([P, ROW], f32, tag="pse")
                            nc.tensor.matmul(out=ptab[:, :], lhsT=xtb[:, :],
                                             rhs=w_next[k + 1][:, :],
                                             start=True, stop=True)
                            write_stripe(k + 1, li, ptab)
                        else:
                            z_t = ep.tile([P, DOUT], f32, tag="z")
                            nc.scalar.activation(out=z_t[:, :], in_=agg[:, :],
                                                 func=Act.Copy, scale=r_t[:, :])
                            ez = ep.tile([P, DOUT], f32, tag="ez")
                            ssum = ep.tile([P, 1], f32, tag="ssum")
                            nc.scalar.activation(out=ez[:, :], in_=z_t[:, :],
                                                 func=Act.Exp,
                                                 accum_out=ssum[:, :])
                            ls = ep.tile([P, 1], f32, tag="ls")
                            nc.scalar.activation(out=ls[:, :], in_=ssum[:, :],
                                                 func=Act.Ln)
                            outf = ep.tile([P, DOUT], bf16, tag="outf")
                            nc.vector.tensor_scalar(out=outf[:, :],
                                                    in0=z_t[:, :],
                                                    scalar1=ls[:, :],
                                                    scalar2=None,
                                                    op0=Alu.subtract)
                            nc.sync.dma_start(
                                out=out_d[li * P:(li + 1) * P, :],
                                in_=outf[:, :])
                    assert i16cur == N16
                    if k < n_layers - 1:
                        gather_table(k + 1)

            for rep in range(repeat):
                one_pass(rep)

    nc.compile()
    return nc


# ----------------------------------------------------------------------------
# Entry point
# ----------------------------------------------------------------------------
def _make_inputs(pre, W_list):
    import ml_dtypes
    bf = ml_dtypes.bfloat16

    SLICE, BPD, TOTD = pre["SLICE"], pre["BPD"], pre["TOTD"]
    offs, CBLOB, NI16p = _blob_layout(SLICE, BPD, TOTD)
    C2 = NI16p // 8
    n16 = (TOTD * P) // 16

    ws = []
    for (W, asr, ads) in W_list:
        W = np.asarray(W, dtype=np.float32)
        din = W.shape[0]
        waug = np.zeros((din, ROW), dtype=np.float32)
        waug[:, :64] = W
        waug[:, 64] = W @ np.asarray(asr, np.float32)
        waug[:, 65] = W @ np.asarray(ads, np.float32)
        ws.append(waug)
    w0b = ws[0].astype(bf)                       # [128, 128]
    w12b = np.zeros((P, 2 * ROW), dtype=bf)
    w12b[0:HID, 0:ROW] = ws[1].astype(bf)
    w12b[0:HID, ROW:2 * ROW] = ws[2].astype(bf)
    identb = np.eye(P, dtype=np.float32).astype(bf)
    iota = np.broadcast_to(np.arange(P, dtype=np.float32), (P, P))

    in_maps = []
    for d in range(NCORES):
        blob = np.zeros((P, CBLOB), dtype=np.float32)
        xTo_bf = pre["xT"][:, d * SLICE:(d + 1) * SLICE].astype(bf)
        blob[:, offs["xTo"]:offs["xTo"] + SLICE // 2] = \
            np.ascontiguousarray(xTo_bf).view(np.float32)
        idxp = np.zeros((16, NI16p), dtype=np.int16)
        idxp[:, :n16] = pre["idx16"][d]
        arr = np.zeros((P, C2), dtype=np.int16)
        for a in range(8):
            arr[16 * a:16 * (a + 1), :] = idxp[:, a * C2:(a + 1) * C2]
        blob[:, offs["idx"]:offs["idx"] + C2 // 2] = arr.view(np.float32)
        blob[:, offs["degA"]:offs["degA"] + BPD] = pre["degA"][d]
        blob[:, offs["degB"]:offs["degB"] + BPD] = pre["degB"][d]
        blob[:, offs["w0"]:offs["w0"] + ROW // 2] = \
            np.ascontiguousarray(w0b).view(np.float32)
        blob[:, offs["w12"]:offs["w12"] + ROW] = \
            np.ascontiguousarray(w12b).view(np.float32)
        blob[:, offs["identb"]:offs["identb"] + P // 2] = \
            np.ascontiguousarray(identb).view(np.float32)
        blob[:, offs["iota"]:offs["iota"] + P] = iota
        in_maps.append({"blob": blob})
    return in_maps


def kernel(x, edge_index, W0, a_src0, a_dst0, W1, a_src1, a_dst1,
           W2, a_src2, a_dst2):
    global LAST_EXEC_NS, LAST_RESULT
    from concourse.bass_utils import run_bass_kernel_spmd

    x = np.asarray(x, dtype=np.float32)
    pre = _preprocess(x, np.asarray(edge_index))

    key = (pre["NPAD"], pre["BPD"], tuple(pre["DA"]), tuple(pre["DB"]))
    if key not in _CACHE:
        _CACHE[key] = _build_program(pre["NPAD"], pre["BPD"], pre["DA"],
                                     pre["DB"], pre["TOTD"])
    nc = _CACHE[key]

    in_maps = _make_inputs(pre, ((W0, a_src0, a_dst0), (W1, a_src1, a_dst1),
                                 (W2, a_src2, a_dst2)))
    trace = bool(int(os.environ.get("GAT_TRACE", "0")))
    res = run_bass_kernel_spmd(nc, in_maps, list(range(NCORES)), trace=trace)
    LAST_EXEC_NS = res.exec_time_ns
    LAST_RESULT = res

    out = np.zeros((pre["N"], DOUT), dtype=np.float32)
    SLICE = pre["SLICE"]
    for d in range(NCORES):
        od = np.asarray(res.results[d]["out"], dtype=np.float32)
        nodes = pre["node_of_slot"][d * SLICE:(d + 1) * SLICE]
        ok = nodes >= 0
        out[nodes[ok]] = od[ok]
    return out
